# revision 9
# baseline (speedup 1.0000x reference)
"""Trainium2 Bass kernel for nn_JointLearningModel (coref-style joint model).

Sharding: the 384x384 pair grid is split by rows across 8 NeuronCores,
row i -> core i%8 (modulo sharding). Only the lower triangle j < i is
computed: with modulo sharding, local row k on any core has global index
8k+d (d<8), so a core-independent static column extent C_k =
roundup(8k+7, 32) covers every core's true extent and the per-core PE
work is identical (perfect balance). Columns beyond the true extent are
killed by the causal mask (-1e4) before the row softmax, which
underflows to exactly 0 in fp32.

all_mention_representations are replicated (each core gathers them via
indirect DMA); params replicated; the scalar loss is computed per-core
over its row set (+ its slice of the character CE) and summed on host.

Pipelining: the per-batch W3 score reduction on PE is deferred by one
batch so PE never waits on the scalar engine's relu output.
"""

import numpy as np
import ml_dtypes

import concourse.bass as bass
import concourse.mybir as mybir
import concourse.tile as tile
from concourse import bacc
from concourse.bass_utils import run_bass_kernel_spmd

F32 = mybir.dt.float32
BF16 = mybir.dt.bfloat16
I32 = mybir.dt.int32
AF = mybir.ActivationFunctionType
OP = mybir.AluOpType

B, L, H, M = 8, 512, 768, 383
N = M + 1          # 384 rows/cols of the pair grid
NC_ = 8            # cores
R = N // NC_       # 48 rows per core
HC = H // 128      # 6 k-chunks of the hidden dim
NEG = -10000.0
NSRC = B * L + 400 + 1 + 1   # seq rows + speaker rows + dummy + zeros row
DUMMY_ROW = B * L + 400
ZERO_ROW = DUMMY_ROW + 1
FMAX = 512         # PSUM bank capacity in fp32 elements per partition

_CACHE = {}
LAST_RESULT = None


def _extent(k):
    """Static column extent for local row k (covers 8k+d for all d<8)."""
    return min(N, 32 * ((8 * k + 7 + 31) // 32))


def _batch_plan():
    """Greedy pack consecutive local rows into batches with G*C <= FMAX."""
    plan = []
    k = 0
    while k < R:
        g = 1
        c = _extent(k)
        while k + g < R:
            c2 = _extent(k + g)
            if (g + 1) * c2 <= FMAX:
                g += 1
                c = c2
            else:
                break
        plan.append((k, g, c))
        k += g
    return plan


BATCHES = _batch_plan()


def _build_program():
    nc = bacc.Bacc(
        "TRN2", target_bir_lowering=False, debug=False, enable_asserts=False
    )

    def din(name, shape, dt):
        return nc.dram_tensor(name, list(shape), dt, kind="ExternalInput")

    # mention representations, pre-gathered and pre-transposed on host
    rT_in = din("rT_in", [128, HC, N], BF16)   # all_reps.T chunked
    rTl_in = din("rTl_in", [128, HC, R], BF16)  # local rows (per-core)
    # pair MLP weights
    waT = din("waT", [128, HC, H], BF16)      # waT[p,ci,o] = Wa.T[ci*128+p, o]
    wbT = din("wbT", [128, HC, H], BF16)
    w2T = din("w2T", [128, HC, H // 2], BF16)
    w3c = din("w3c", [128, 3], BF16)
    b1c = din("b1c", [128, HC], F32)
    b2c = din("b2c", [128, 3], F32)
    # mention-score MLP
    wm1T = din("wm1T", [128, HC, H // 2], BF16)
    bm1c = din("bm1c", [128, 3], F32)
    wm2T = din("wm2T", [128, 3, H // 4], BF16)
    bm2c = din("bm2c", [128, 2], F32)
    wm3c = din("wm3c", [128, 2], BF16)
    # character head
    wc1T = din("wc1T", [128, HC, H // 2], BF16)
    bc1c = din("bc1c", [128, 3], F32)
    wc2T = din("wc2T", [128, 3, 18], BF16)
    bc2r = din("bc2r", [1, 18], F32)
    # per-core loss plumbing
    maskb = din("maskb", [R, N], F32)
    multb = din("multb", [R, N], F32)
    wnll = din("wnll", [R, 1], F32)
    oneh = din("oneh", [R, 18], F32)
    wch = din("wch", [R, 1], F32)

    loss = nc.dram_tensor("loss", [1, 1], F32, kind="ExternalOutput")

    with tile.TileContext(nc) as tc:
        with tc.tile_pool(name="const", bufs=1) as cp:
            # ---- resident tiles (DMA'd once) ----
            def load(name, h):
                t = cp.tile(list(h.shape), h.dtype, name=name)
                nc.sync.dma_start(out=t[:], in_=h.ap())
                return t

            rT = load("rT", rT_in)
            rTl = load("rTl", rTl_in)
            waT_sb = load("waT_sb", waT)
            wbT_sb = load("wbT_sb", wbT)
            w2T_sb = load("w2T_sb", w2T)
            w3c_sb = load("w3c_sb", w3c)
            b1c_sb = load("b1c_sb", b1c)
            b2c_sb = load("b2c_sb", b2c)
            wm1T_sb = load("wm1T_sb", wm1T)
            bm1c_sb = load("bm1c_sb", bm1c)
            wm2T_sb = load("wm2T_sb", wm2T)
            bm2c_sb = load("bm2c_sb", bm2c)
            wm3c_sb = load("wm3c_sb", wm3c)
            wc1T_sb = load("wc1T_sb", wc1T)
            bc1c_sb = load("bc1c_sb", bc1c)
            wc2T_sb = load("wc2T_sb", wc2T)
            bc2r_sb = load("bc2r_sb", bc2r)
            maskb_sb = load("maskb_sb", maskb)
            multb_sb = load("multb_sb", multb)
            wnll_sb = load("wnll_sb", wnll)
            oneh_sb = load("oneh_sb", oneh)
            wch_sb = load("wch_sb", wch)

            one1 = cp.tile([1, R], F32)
            nc.vector.memset(one1[:], 1.0)

            # outputs of the preamble, used by the main loop / epilogue
            at_sb = cp.tile([128, HC, N], BF16)    # A.T   (bf16)
            bb_sb = cp.tile([128, HC, R], F32)     # Bm.T + b1, local rows
            sblk = cp.tile([R, N], F32)            # assembled pair scores
            nc.vector.memset(sblk[:], 0.0)

            # ---------- preamble matmuls: A.T, Bb ----------
            with tc.tile_pool(name="pre_ps", bufs=2, space="PSUM") as pp:
                for co in range(HC):
                    pa = pp.tile([128, N], F32, tag="big", name=f"pa_{co}")
                    for ci in range(HC):
                        nc.tensor.matmul(
                            out=pa[:],
                            lhsT=waT_sb[:, ci, co * 128 : (co + 1) * 128],
                            rhs=rT[:, ci, :],
                            start=(ci == 0),
                            stop=(ci == HC - 1),
                        )
                    nc.scalar.copy(out=at_sb[:, co, :], in_=pa[:])
                for co in range(HC):
                    pb = pp.tile([128, R], F32, tag="small", name=f"pb_{co}")
                    for ci in range(HC):
                        nc.tensor.matmul(
                            out=pb[:],
                            lhsT=wbT_sb[:, ci, co * 128 : (co + 1) * 128],
                            rhs=rTl[:, ci, :],
                            start=(ci == 0),
                            stop=(ci == HC - 1),
                        )
                    nc.vector.tensor_scalar(
                        out=bb_sb[:, co, :],
                        in0=pb[:],
                        scalar1=b1c_sb[:, co : co + 1],
                        scalar2=None,
                        op0=OP.add,
                    )

            # ---------- main loop: batched triangular pair grid ----------
            with (
                tc.tile_pool(name="lp_sb", bufs=1) as lsb,
                tc.tile_pool(name="lp_ps", bufs=2, space="PSUM") as lps,
                tc.tile_pool(name="sr_ps", bufs=2, space="PSUM") as sps,
            ):
                pend = None  # (h2s tiles, F, k0, G, C) awaiting W3 reduce

                def flush_pend():
                    h2s, F, k0, G, C = pend
                    sr = sps.tile([1, FMAX], F32, tag="srow", name=f"sr_{k0}",
                                  bufs=2)
                    for hb in range(3):
                        nc.tensor.matmul(
                            out=sr[:, :F],
                            lhsT=w3c_sb[:, hb : hb + 1],
                            rhs=h2s[hb][:, :F],
                            start=(hb == 0),
                            stop=(hb == 2),
                        )
                    srow = lsb.tile([1, FMAX], F32, tag="srow_sb",
                                    name=f"srow_{k0}", bufs=2)
                    nc.scalar.copy(out=srow[:, :F], in_=sr[:, :F])
                    nc.sync.dma_start(
                        out=sblk[k0 : k0 + G, 0:C], in_=srow[0:1, 0:F]
                    )

                for (k0, G, C) in BATCHES:
                    F = G * C
                    h1 = lsb.tile([128, HC, FMAX], BF16, tag="h1",
                                  name=f"h1_{k0}", bufs=3)
                    for c in range(HC):
                        for g in range(G):
                            nc.vector.tensor_scalar(
                                out=h1[:, c, g * C : (g + 1) * C],
                                in0=at_sb[:, c, 0:C],
                                scalar1=bb_sb[:, c, k0 + g : k0 + g + 1],
                                scalar2=0.0,
                                op0=OP.add,
                                op1=OP.max,
                            )
                    h2s = []
                    for hb in range(3):
                        ph = lps.tile([128, FMAX], F32, tag=f"h2_{hb}",
                                      name=f"ph_{k0}_{hb}")
                        for c in range(HC):
                            nc.tensor.matmul(
                                out=ph[:, :F],
                                lhsT=w2T_sb[:, c, hb * 128 : (hb + 1) * 128],
                                rhs=h1[:, c, 0:F],
                                start=(c == 0),
                                stop=(c == HC - 1),
                            )
                        hs = lsb.tile([128, FMAX], BF16, tag=f"h2s_{hb}",
                                      name=f"hs_{k0}_{hb}", bufs=2)
                        nc.scalar.activation(
                            out=hs[:, :F], in_=ph[:, :F], func=AF.Relu,
                            bias=b2c_sb[:, hb : hb + 1],
                        )
                        h2s.append(hs)
                    if pend is not None:
                        flush_pend()
                    pend = (h2s, F, k0, G, C)
                flush_pend()

            # ---------- epilogue: ms MLP, masked row-softmax, char CE ----------
            with (
                tc.tile_pool(name="ep_sb", bufs=1) as ep,
                tc.tile_pool(name="ep_ps", bufs=2, space="PSUM") as eps,
            ):
                # mention score MLP (768 -> 384 -> 192 -> 1) on all N mentions
                ms1 = ep.tile([128, 3, N], BF16)
                for co in range(3):
                    pm = eps.tile([128, N], F32, tag="big", name=f"pm_{co}")
                    for ci in range(HC):
                        nc.tensor.matmul(
                            out=pm[:],
                            lhsT=wm1T_sb[:, ci, co * 128 : (co + 1) * 128],
                            rhs=rT[:, ci, :],
                            start=(ci == 0),
                            stop=(ci == HC - 1),
                        )
                    nc.scalar.activation(
                        out=ms1[:, co, :],
                        in_=pm[:],
                        func=AF.Relu,
                        bias=bm1c_sb[:, co : co + 1],
                    )
                ms2 = ep.tile([128, 2, N], BF16)
                for co, sz in enumerate((128, 64)):
                    pm2 = eps.tile([128, N], F32, tag="big", name=f"pm2_{co}")
                    for ci in range(3):
                        nc.tensor.matmul(
                            out=pm2[:sz, :],
                            lhsT=wm2T_sb[:, ci, co * 128 : co * 128 + sz],
                            rhs=ms1[:, ci, :],
                            start=(ci == 0),
                            stop=(ci == 2),
                        )
                    nc.scalar.activation(
                        out=ms2[:sz, co, :],
                        in_=pm2[:sz, :],
                        func=AF.Relu,
                        bias=bm2c_sb[:sz, co : co + 1],
                    )
                pms = eps.tile([1, N], F32, tag="small", bufs=1)
                nc.tensor.matmul(
                    out=pms[:], lhsT=wm3c_sb[:, 0:1], rhs=ms2[:, 0, :],
                    start=True, stop=False,
                )
                nc.tensor.matmul(
                    out=pms[:], lhsT=wm3c_sb[:64, 1:2], rhs=ms2[:64, 1, :],
                    start=False, stop=True,
                )
                ms_sb = ep.tile([1, N], F32)
                nc.vector.tensor_copy(out=ms_sb[:], in_=pms[:])
                # broadcast ms over the 48 rows and add the causal mask
                mskms = ep.tile([R, N], F32)
                pbc = eps.tile([R, N], F32, tag="big")
                nc.tensor.matmul(
                    out=pbc[:], lhsT=one1[:], rhs=ms_sb[:], start=True, stop=True
                )
                nc.vector.tensor_tensor(
                    out=mskms[:], in0=pbc[:], in1=maskb_sb[:], op=OP.add
                )

                x = ep.tile([R, N], F32)
                nc.vector.tensor_tensor(out=x[:], in0=sblk[:], in1=mskms[:], op=OP.add)
                rm = ep.tile([R, 1], F32)
                nc.vector.tensor_reduce(
                    out=rm[:], in_=x[:], axis=mybir.AxisListType.X, op=OP.max
                )
                nrm = ep.tile([R, 1], F32)
                nc.vector.tensor_scalar_mul(nrm[:], rm[:], -1.0)
                pexp = ep.tile([R, N], F32)
                z = ep.tile([R, 1], F32)
                nc.scalar.activation(
                    out=pexp[:], in_=x[:], func=AF.Exp, bias=nrm[:, 0:1],
                    accum_out=z[:],
                )
                escr = ep.tile([R, N], F32)
                nc.vector.tensor_tensor(
                    out=escr[:], in0=pexp[:], in1=multb_sb[:], op=OP.mult
                )
                e = ep.tile([R, 1], F32)
                nc.vector.tensor_reduce(
                    out=e[:], in_=escr[:], axis=mybir.AxisListType.X, op=OP.add
                )
                lz = ep.tile([R, 1], F32)
                nc.scalar.activation(out=lz[:], in_=z[:], func=AF.Ln)
                le = ep.tile([R, 1], F32)
                nc.scalar.activation(out=le[:], in_=e[:], func=AF.Ln)
                tnll = ep.tile([R, 1], F32)
                nc.vector.tensor_tensor(
                    out=tnll[:], in0=lz[:], in1=le[:], op=OP.subtract
                )
                pl = eps.tile([1, 1], F32, tag="loss", bufs=1)
                nc.tensor.matmul(
                    out=pl[:], lhsT=tnll[:, 0:1], rhs=wnll_sb[:], start=True,
                    stop=False,
                )
                # character head on local mentions
                c1 = ep.tile([128, 3, R], BF16)
                for co in range(3):
                    pc = eps.tile([128, R], F32, tag="pc", name=f"pc_{co}")
                    for ci in range(HC):
                        nc.tensor.matmul(
                            out=pc[:],
                            lhsT=wc1T_sb[:, ci, co * 128 : (co + 1) * 128],
                            rhs=rTl[:, ci, :],
                            start=(ci == 0),
                            stop=(ci == HC - 1),
                        )
                    nc.scalar.activation(
                        out=c1[:, co, :], in_=pc[:], func=AF.Relu,
                        bias=bc1c_sb[:, co : co + 1],
                    )
                plg = eps.tile([R, 18], F32, tag="lg", bufs=1)
                for co in range(3):
                    nc.tensor.matmul(
                        out=plg[:], lhsT=c1[:, co, :], rhs=wc2T_sb[:, co, :],
                        start=(co == 0), stop=False,
                    )
                nc.tensor.matmul(
                    out=plg[:], lhsT=one1[:], rhs=bc2r_sb[:], start=False, stop=True
                )
                cm = ep.tile([R, 1], F32)
                nc.vector.tensor_reduce(
                    out=cm[:], in_=plg[:], axis=mybir.AxisListType.X, op=OP.max
                )
                ncm = ep.tile([R, 1], F32)
                nc.vector.tensor_scalar_mul(ncm[:], cm[:], -1.0)
                cexp = ep.tile([R, 18], F32)
                cz = ep.tile([R, 1], F32)
                nc.scalar.activation(
                    out=cexp[:], in_=plg[:], func=AF.Exp, bias=ncm[:, 0:1],
                    accum_out=cz[:],
                )
                cscr = ep.tile([R, 18], F32)
                nc.vector.tensor_tensor(
                    out=cscr[:], in0=plg[:], in1=oneh_sb[:], op=OP.mult
                )
                sl = ep.tile([R, 1], F32)
                nc.vector.tensor_reduce(
                    out=sl[:], in_=cscr[:], axis=mybir.AxisListType.X, op=OP.add
                )
                lcz = ep.tile([R, 1], F32)
                nc.scalar.activation(out=lcz[:], in_=cz[:], func=AF.Ln)
                cev = ep.tile([R, 1], F32)
                nc.vector.tensor_tensor(
                    out=cev[:], in0=lcz[:], in1=cm[:], op=OP.add
                )
                nc.vector.tensor_tensor(
                    out=cev[:], in0=cev[:], in1=sl[:], op=OP.subtract
                )
                nc.tensor.matmul(
                    out=pl[:], lhsT=cev[:, 0:1], rhs=wch_sb[:], start=False,
                    stop=True,
                )
                lout = ep.tile([1, 1], F32)
                nc.vector.tensor_copy(out=lout[:], in_=pl[:])
                nc.sync.dma_start(out=loss.ap(), in_=lout[:])

    nc.compile()
    return nc


def _chunk_cols(w):
    """[K, O] -> [128, K//128, O]  (partition-chunked contraction dim)."""
    k, o = w.shape
    return np.ascontiguousarray(w.reshape(k // 128, 128, o).transpose(1, 0, 2))


def _chunk_vec(v, ncol):
    """[C] -> [128, ncol] column-chunks (zero padded)."""
    out = np.zeros((128, ncol), np.float32)
    for c in range(ncol):
        seg = v[c * 128 : (c + 1) * 128]
        out[: len(seg), c] = seg
    return out


def _prep_in_maps(inputs):
    bf = ml_dtypes.bfloat16

    seq = np.asarray(inputs["sequence_output"], np.float32)
    spk = np.asarray(inputs["speaker_emb"], np.float32)
    dummy = np.asarray(inputs["dummy_emb"], np.float32)

    seg = np.asarray(inputs["mentions_seg"]).astype(np.int64)
    mstart = np.asarray(inputs["mention_start"]).astype(np.int64)
    mend = np.asarray(inputs["mention_end"]).astype(np.int64)
    sid = np.asarray(inputs["speaker_ids"]).astype(np.int64)[seg, mstart]
    mention_reps = seq[seg, mstart] + seq[seg, mend] + spk[sid]  # [M, H] f32
    all_reps = np.concatenate([dummy, mention_reps], axis=0)     # [N, H]
    # rT[p, c, m] = all_reps[m, c*128+p]
    rT_np = np.ascontiguousarray(
        all_reps.reshape(N, HC, 128).transpose(2, 1, 0)
    ).astype(bf)                                                 # [128, HC, N]

    W_pair1 = np.asarray(inputs["W_pair1"], np.float32)
    waT = _chunk_cols(np.ascontiguousarray(W_pair1[:, :H].T)).astype(bf)
    wbT = _chunk_cols(np.ascontiguousarray(W_pair1[:, H:].T)).astype(bf)
    w2T = _chunk_cols(
        np.ascontiguousarray(np.asarray(inputs["W_pair2"], np.float32).T)
    ).astype(bf)
    w3c = _chunk_vec(np.asarray(inputs["W_pair3"], np.float32)[0], 3).astype(bf)
    b1c = _chunk_vec(np.asarray(inputs["b_pair1"], np.float32), HC)
    b2c = _chunk_vec(np.asarray(inputs["b_pair2"], np.float32), 3)
    wm1T = _chunk_cols(
        np.ascontiguousarray(np.asarray(inputs["W_m1"], np.float32).T)
    ).astype(bf)
    bm1c = _chunk_vec(np.asarray(inputs["b_m1"], np.float32), 3)
    wm2T = _chunk_cols(
        np.ascontiguousarray(np.asarray(inputs["W_m2"], np.float32).T)
    ).astype(bf)
    bm2c = _chunk_vec(np.asarray(inputs["b_m2"], np.float32), 2)
    wm3c = _chunk_vec(np.asarray(inputs["W_m3"], np.float32)[0], 2).astype(bf)
    wc1T = _chunk_cols(
        np.ascontiguousarray(np.asarray(inputs["W_c1"], np.float32).T)
    ).astype(bf)
    bc1c = _chunk_vec(np.asarray(inputs["b_c1"], np.float32), 3)
    wc2T = _chunk_cols(
        np.ascontiguousarray(np.asarray(inputs["W_c2"], np.float32).T)
    ).astype(bf)
    bc2r = np.asarray(inputs["b_c2"], np.float32).reshape(1, 18)

    link_first = np.asarray(inputs["link_first"]).astype(np.int64)
    link_second = np.asarray(inputs["link_second"]).astype(np.int64)
    label = np.asarray(inputs["character_label"]).astype(np.int64)

    mult = np.zeros((N, N), np.float32)
    np.add.at(mult, (link_second, link_first), 1.0)
    has_link = mult.sum(axis=1) > 0
    wnll_full = ((np.arange(N) >= 1) & has_link).astype(np.float32)
    mult[~has_link, 0] = 1.0  # keep log(E) finite; weight is 0 there

    mask_full = np.where(
        np.arange(N)[None, :] >= np.arange(N)[:, None], np.float32(NEG), 0.0
    ).astype(np.float32)

    oneh_full = np.zeros((N, 18), np.float32)
    wch_full = np.zeros(N, np.float32)
    oneh_full[np.arange(1, N), label] = 1.0
    wch_full[1:] = 1.0

    shared = dict(
        rT_in=rT_np,
        waT=waT, wbT=wbT, w2T=w2T, w3c=w3c, b1c=b1c, b2c=b2c,
        wm1T=wm1T, bm1c=bm1c, wm2T=wm2T, bm2c=bm2c, wm3c=wm3c,
        wc1T=wc1T, bc1c=bc1c, wc2T=wc2T, bc2r=bc2r,
    )
    in_maps = []
    for d in range(NC_):
        rows = np.arange(R) * NC_ + d      # modulo sharding: row 8k+d
        m = dict(shared)
        m["rTl_in"] = np.ascontiguousarray(rT_np[:, :, rows])
        m["maskb"] = np.ascontiguousarray(mask_full[rows])
        m["multb"] = np.ascontiguousarray(mult[rows])
        m["wnll"] = np.ascontiguousarray(wnll_full[rows]).reshape(R, 1)
        m["oneh"] = np.ascontiguousarray(oneh_full[rows])
        m["wch"] = np.ascontiguousarray(wch_full[rows]).reshape(R, 1)
        in_maps.append(m)
    return in_maps


def kernel(**inputs):
    global LAST_RESULT
    in_maps = _prep_in_maps(inputs)

    if "nc" not in _CACHE:
        _CACHE["nc"] = _build_program()
    nc = _CACHE["nc"]

    res = run_bass_kernel_spmd(nc, in_maps, core_ids=list(range(NC_)))
    LAST_RESULT = res
    total = np.float32(0.0)
    for d in range(NC_):
        total += np.float32(res.results[d]["loss"][0, 0])
    return np.asarray(total, dtype=np.float32)


if __name__ == "__main__":
    import reference

    inputs = {k: np.asarray(v) for k, v in reference.setup_inputs().items()}
    out = kernel(**inputs)
    print("kernel out:", out)
